# revision 1
# baseline (speedup 1.0000x reference)
"""Trainium2 Bass kernel for nn_MultiHeadAttention_78331613544953.

Reference computation (B=2, S=2048, D=1024, H=16, HD=64):
    qkv = x @ W_qkv + b_qkv                       # [B,S,3D]
    q,k,v per head (head h owns columns [h*192,(h+1)*192) of W_qkv);
    scores = q @ k.T / 8 + causal_mask
    attn = softmax(scores); values = attn @ v     # [B,H,S,HD]
    values = values.reshape(B, S, H*HD)           # "faithful" raw reshape
    out = values @ W_out + b_out

The raw reshape maps head h's output rows to out rows [h*128,(h+1)*128):
    values_resh[h*128 + s//16, (s%16)*64 + hd] = values[h, s, hd]

Sharding: 8 cores = 2 batches x 4 head-groups (4 heads each). Core c handles
batch c//4, heads [4*(c%4), 4*(c%4)+4) and produces out rows
[b, (c%4)*512 : (c%4)*512+512, :].

Per-core kernel strategy (all matmuls bf16 inputs, fp32 PSUM accumulate):
  - x [2048,1024] f32 -> bf16 cast-DMA -> xbar DMA-transpose -> xT [d, s]
  - qT/kT per head-pair [128(2 heads x hd), 2048] = W.T @ x.T directly
  - v4 [s-block, 4*64] natural; vext per head [k-block, 65] = [v+bv | ones]
  - scoresT[k,q] blocks = kT_h-slice.T @ qT panel (2 heads packed in the PE
    array via tile_position row-groups, separate PSUM banks); exp via ACT
    (scale=1/8 folded; no max subtraction -- logits are O(2.5)); causal
    handled by computing only lower k-blocks, multiplying diagonal blocks by
    a triangular 0/1 mask, zeroing the fully-masked half-block.
  - valuesT[hd,q] += vext.T @ attnT per k-block; row 64 = softmax sums
    (ones column of vext). Normalize: reciprocal of sums row, broadcast via
    a DRAM bounce, multiply -> bf16.
  - out rows = sum_j valuesT[:, j::16].T @ W_out[j*64:(j+1)*64]: the raw
    reshape scramble is just a strided AP slice of valuesT.
The two head pairs' attention panels are interleaved (pair 1 skewed one
panel behind pair 0) so scalar-engine exp of one pair overlaps tensor-engine
work of the other.
"""
import functools
import numpy as np

import concourse.bass as bass
import concourse.mybir as mybir
import concourse.tile as tile
from concourse import bacc, bass_utils

F32 = mybir.dt.float32
BF16 = mybir.dt.bfloat16
AF = mybir.ActivationFunctionType

S = 2048
D = 1024
HD = 64
HPC = 4          # heads per core
NKT = 8          # 128-row k-tiles in D
NSB = 16         # 128-row s-blocks in S
QC = 256         # q panel width for attention
NQP = S // QC    # 8 q panels
NCORES = 8


def build_nc(dbg=False):
    nc = bacc.Bacc("TRN2", debug=False)

    X = nc.dram_tensor("X", [S, D], F32, kind="ExternalInput").ap()
    WQ = nc.dram_tensor("WQ", [D, HPC * HD], F32, kind="ExternalInput").ap()
    WK = nc.dram_tensor("WK", [D, HPC * HD], F32, kind="ExternalInput").ap()
    WV = nc.dram_tensor("WV", [D, HPC * HD], F32, kind="ExternalInput").ap()
    BQ = nc.dram_tensor("BQ", [HPC * HD], F32, kind="ExternalInput").ap()
    BK = nc.dram_tensor("BK", [HPC * HD], F32, kind="ExternalInput").ap()
    BV = nc.dram_tensor("BV", [HPC * HD], F32, kind="ExternalInput").ap()
    WO = nc.dram_tensor("WO", [D, D], F32, kind="ExternalInput").ap()
    BO = nc.dram_tensor("BO", [D], F32, kind="ExternalInput").ap()
    OUT = nc.dram_tensor("OUT", [HPC * 128, D], F32, kind="ExternalOutput").ap()
    if dbg:
        D_QT = nc.dram_tensor("D_QT", [2, 128, S], F32, kind="ExternalOutput").ap()
        D_KT = nc.dram_tensor("D_KT", [2, 128, S], F32, kind="ExternalOutput").ap()
        D_VE = nc.dram_tensor("D_VE", [HPC, 128, NSB, HD + 1], F32, kind="ExternalOutput").ap()
        D_VT = nc.dram_tensor("D_VT", [HPC, 64, S], F32, kind="ExternalOutput").ap()
        D_SUM = nc.dram_tensor("D_SUM", [HPC, NQP, QC], F32, kind="ExternalOutput").ap()

    with tile.TileContext(nc) as tc:
        with (
            tc.tile_pool(name="const", bufs=1) as const,
            tc.tile_pool(name="xstage", bufs=1) as xstage,
            tc.tile_pool(name="work", bufs=2) as work,
            tc.tile_pool(name="dscr", bufs=2, space="DRAM") as dscr,
        ):
            # ---- ingestion ----
            # The DMA xbar serializes on every copy<->transpose mode switch
            # (known HW bug workaround in Tile), so x loads and transposes are
            # emitted in alternating GROUPS: each transpose batch only waits
            # for its own group's copies.
            xT = xstage.tile([128, NKT, S], BF16, tag="xT")
            with (
                tc.tile_pool(name="xbf", bufs=1) as xbfp,
                tc.tile_pool(name="stg", bufs=1) as stg,
            ):
                xr = X.rearrange("(n p) d -> p n d", p=128)

                bqk = const.tile([128, 2, 2], F32, tag="bqk")  # [:, pair, {q,k}]
                for i in range(2):
                    nc.sync.dma_start(out=bqk[:, i, 0:1], in_=BQ[i * 128:(i + 1) * 128].unsqueeze(1))
                    nc.sync.dma_start(out=bqk[:, i, 1:2], in_=BK[i * 128:(i + 1) * 128].unsqueeze(1))

                x_sbs = {}

                def load_x(sb):
                    # f32 loads alternate across the two HWDGE rings (SP/ACT);
                    # the bf16 cast runs on GpSimd, which is otherwise idle
                    st = stg.tile([128, D], F32, tag="stgx", bufs=6, name=f"stgx{sb}")
                    eng = nc.sync if sb % 2 == 0 else nc.scalar
                    eng.dma_start(out=st, in_=xr[:, sb, :])
                    x_sb = xbfp.tile([128, D], BF16, tag="x_sb", bufs=8, name=f"x_sb{sb}")
                    # spread the bf16 casts: ACT is idle before attention
                    # starts, afterwards alternate GpSimd/DVE
                    if sb < 6:
                        nc.scalar.copy(x_sb, st)
                    elif sb % 2 == 0:
                        nc.gpsimd.tensor_copy(x_sb, st)
                    else:
                        nc.vector.tensor_copy(x_sb, st)
                    x_sbs[sb] = x_sb

                def trans_x(sb):
                    # one 3D xbar transpose per s-block:
                    # xT[p, t, sb*128+c] = x_sb[c, t*128+p]
                    # all transposes stay on one HWDGE ring: the xbar-mode
                    # (copy vs transpose) workaround assumes a single stream
                    nc.sync.dma_start_transpose(
                        xT[:, :, sb * 128:(sb + 1) * 128], x_sbs[sb])

                wq = const.tile([128, NKT, HPC * HD], BF16, tag="wq")
                wk = const.tile([128, NKT, HPC * HD], BF16, tag="wk")
                wv = const.tile([128, NKT, HPC * HD], BF16, tag="wv")

                def load_w(dst, SRC):
                    # HWDGE f32 load + DVE cast: keeps the Q7 SWDGE descriptor
                    # queue free for GpSimd cast work
                    st = stg.tile([128, 2048], F32, tag="stgw", bufs=2, name=f"stgw_{SRC.name}")
                    stv = st.rearrange("p (t c) -> p t c", t=NKT)
                    nc.sync.dma_start(out=stv, in_=SRC.rearrange("(t p) c -> p t c", p=128))
                    nc.vector.tensor_copy(dst, stv)

                for sb in range(8):
                    load_x(sb)
                load_w(wq, WQ)
                load_w(wk, WK)
                for sb in range(8):
                    trans_x(sb)
                for sb in range(8, NSB):
                    load_x(sb)
                for sb in range(8, NSB):
                    trans_x(sb)

                # remaining constants (after the last transpose so their copies
                # don't stall the xbar)
                nc.gpsimd.dma_start(out=wv, in_=WV.rearrange("(t p) c -> p t c", p=128))
                # W_out as 8 K=128 tiles over the scrambled d' axis
                wo128n = const.tile([128, NKT, D], BF16, tag="wo128n")
                nc.gpsimd.dma_start(out=wo128n, in_=WO.rearrange("(t p) c -> p t c", p=128))
                bv_bc = const.tile([128, HPC * HD], F32, tag="bv_bc")
                nc.sync.dma_start(
                    out=bv_bc,
                    in_=bass.AP(tensor=BV.tensor, offset=BV.offset, ap=[[0, 128]] + list(BV.ap)))
                bo_bc = const.tile([128, D], F32, tag="bo_bc")
                nc.sync.dma_start(
                    out=bo_bc,
                    in_=bass.AP(tensor=BO.tensor, offset=BO.offset, ap=[[0, 128]] + list(BO.ap)))

                # triangular keep-mask for transposed diagonal blocks:
                # tri[k, q] = 1.0 if q >= k else 0.0
                tri = const.tile([128, 128], BF16, tag="tri")
                nc.vector.memset(tri, 1.0)
                nc.gpsimd.affine_select(
                    out=tri, in_=tri, compare_op=mybir.AluOpType.is_ge,
                    fill=0.0, base=0, pattern=[[1, 128]], channel_multiplier=-1)

                # ---- fused projection + attention pipeline ----
                # All PSUM pools coexist (exactly 8 banks): pq 1, pv 1,
                # sc{i} 2x2, valT{i} 2 (out-proj borrows the valT banks).
                qT = [xstage.tile([128, S], BF16, tag=f"qT{i}", name=f"qT{i}") for i in range(2)]
                kT = [xstage.tile([128, S], BF16, tag=f"kT{i}", name=f"kT{i}") for i in range(2)]
                vext = [xstage.tile([128, NSB, HD + 1], BF16, tag=f"vext{h}", name=f"vext{h}")
                        for h in range(HPC)]
                valuesT = [xstage.tile([64, S], BF16, tag=f"valuesT{h}", name=f"valuesT{h}")
                           for h in range(HPC)]
                for h in range(HPC):
                    nc.vector.memset(vext[h][:, :, HD:HD + 1], 1.0)

                with (
                    tc.tile_pool(name="attnp", bufs=1) as attnp,
                    tc.tile_pool(name="ps_pq", bufs=1, space="PSUM") as ps_pq,
                    tc.tile_pool(name="ps_pv", bufs=1, space="PSUM") as ps_pv,
                    tc.tile_pool(name="ps_sc", bufs=1, space="PSUM") as ps_sc,
                    tc.tile_pool(name="ps_val", bufs=1, space="PSUM") as ps_val,
                ):
                    attnT = [attnp.tile([128, NSB, QC], BF16, tag=f"attnT{i}_{hh}",
                                        name=f"attnT{i}_{hh}")
                             for i in range(2) for hh in range(2)]

                    # --- projection work units (one PSUM group each) ---
                    def proj_qk_unit(sp, i, which):
                        def emit():
                            w_sb, dst, bcol = ((wq, qT[i], 0), (wk, kT[i], 1))[which]
                            pq = ps_pq.tile([128, 512], F32, tag="pq",
                                            name=f"pq{sp}_{i}_{which}")
                            for kt in range(NKT):
                                nc.tensor.matmul(
                                    pq,
                                    w_sb[:, kt, i * 128:(i + 1) * 128],
                                    xT[:, kt, sp * 512:(sp + 1) * 512],
                                    start=(kt == 0), stop=(kt == NKT - 1))
                            nc.vector.tensor_scalar_add(
                                dst[:, sp * 512:(sp + 1) * 512], pq,
                                bqk[:, i, bcol:bcol + 1])
                        return emit

                    def proj_v_unit(sb):
                        def emit():
                            pv = ps_pv.tile([128, HPC * HD], F32, tag="pv",
                                            name=f"pv{sb}")
                            for kt in range(NKT):
                                nc.tensor.matmul(
                                    pv,
                                    xT[:, kt, sb * 128:(sb + 1) * 128],
                                    wv[:, kt, :],
                                    start=(kt == 0), stop=(kt == NKT - 1))
                            for h in range(HPC):
                                nc.vector.tensor_add(
                                    vext[h][:, sb, 0:HD],
                                    pv[:, h * HD:(h + 1) * HD],
                                    bv_bc[:, h * HD:(h + 1) * HD])
                        return emit

                    def proj_units(sp):
                        us = []
                        for i in range(2):
                            us.append(proj_qk_unit(sp, i, 0))
                            us.append(proj_qk_unit(sp, i, 1))
                        for sb in range(4 * sp, 4 * sp + 4):
                            us.append(proj_v_unit(sb))
                        return us

                    # --- attention panel steps (one head pair): scoresT ->
                    #     exp -> attnT -> valuesT accumulation, software-
                    #     pipelined over kb pairs ---
                    def attn_steps(i, p):
                        kb_max = 2 * p + 1
                        nkbp = p + 1
                        vps = ps_val.tile([HD + 1, 2 * QC], F32, tag=f"valT{i}",
                                          name=f"vps{i}_{p}")
                        at = attnT[2 * i:2 * i + 2]

                        def sc_mms(kbp, sc_t, last):
                            kb0, kb1 = 2 * kbp, 2 * kbp + 1
                            for hh in range(2):
                                lo = hh * 64
                                nc.tensor.matmul(
                                    sc_t[:, hh, 0:QC],
                                    kT[i][lo:lo + 64, kb0 * 128:(kb0 + 1) * 128],
                                    qT[i][lo:lo + 64, p * QC:(p + 1) * QC],
                                    start=True, stop=True, tile_position=(lo, 0))
                                if last:
                                    nc.tensor.matmul(
                                        sc_t[:, hh, QC + 128:2 * QC],
                                        kT[i][lo:lo + 64, kb1 * 128:(kb1 + 1) * 128],
                                        qT[i][lo:lo + 64, p * QC + 128:(p + 1) * QC],
                                        start=True, stop=True, tile_position=(lo, 0))
                                else:
                                    nc.tensor.matmul(
                                        sc_t[:, hh, QC:2 * QC],
                                        kT[i][lo:lo + 64, kb1 * 128:(kb1 + 1) * 128],
                                        qT[i][lo:lo + 64, p * QC:(p + 1) * QC],
                                        start=True, stop=True, tile_position=(lo, 0))

                        first_mm = [None]

                        def consume(kbp, sc_t, last):
                            kb0, kb1 = 2 * kbp, 2 * kbp + 1
                            for hh in range(2):
                                if not last:
                                    nc.scalar.activation(
                                        at[hh][:, kb0:kb0 + 2, :].rearrange("p a b -> p (a b)"),
                                        sc_t[:, hh, :], AF.Exp, bias=0.0, scale=0.125)
                                else:
                                    # kb0 == 2p: diag in left half; kb1 == 2p+1:
                                    # left half fully masked, diag in right half
                                    nc.scalar.activation(
                                        at[hh][:, kb0, :], sc_t[:, hh, 0:QC],
                                        AF.Exp, bias=0.0, scale=0.125)
                                    nc.scalar.activation(
                                        at[hh][:, kb1, 128:QC], sc_t[:, hh, QC + 128:2 * QC],
                                        AF.Exp, bias=0.0, scale=0.125)
                                    nc.vector.memset(at[hh][:, kb1, 0:128], 0.0)
                                    nc.vector.tensor_mul(
                                        at[hh][:, kb0, 0:128], at[hh][:, kb0, 0:128], tri)
                                    nc.vector.tensor_mul(
                                        at[hh][:, kb1, 128:QC], at[hh][:, kb1, 128:QC], tri)
                            for kb in (kb0, kb1):
                                for hh in range(2):
                                    # only the first matmul into the shared bank
                                    # carries start=True: first_mm clears the
                                    # WHOLE bank (probe-verified); the second
                                    # head accumulates onto cleared zeros
                                    mm = nc.tensor.matmul(
                                        vps[:, hh * QC:(hh + 1) * QC],
                                        vext[2 * i + hh][:, kb, :],
                                        at[hh][:, kb, :],
                                        start=(kb == 0 and hh == 0),
                                        stop=(kb == kb_max),
                                        skip_group_check=True)
                                    if kb == 0 and hh == 0:
                                        first_mm[0] = mm
                                    elif kb == 0 and hh == 1:
                                        bass._add_dep_helper(
                                            mm.ins, first_mm[0].ins, sync=False,
                                            reason="bank-clear order: start MM first")

                        pend = [None]
                        for kbp in range(nkbp):
                            last = kbp == nkbp - 1

                            def step(kbp=kbp, last=last):
                                sc_t = ps_sc.tile([128, 2, 2 * QC], F32, tag=f"sc{i}",
                                                  name=f"sc{i}_{p}_{kbp}")
                                sc_mms(kbp, sc_t, last)
                                if pend[0] is not None:
                                    consume(*pend[0])
                                pend[0] = (kbp, sc_t, last)
                            yield step

                        def final():
                            consume(*pend[0])
                            # normalize: values / sums (row 64 of vps)
                            for hh in range(2):
                                h = 2 * i + hh
                                vsb = work.tile([HD + 1, QC], F32, tag="vsb",
                                                name=f"vsb{i}_{p}_{hh}")
                                nc.vector.tensor_copy(vsb, vps[:, hh * QC:(hh + 1) * QC])
                                if dbg:
                                    nc.sync.dma_start(out=D_SUM[h, p].unsqueeze(0),
                                                      in_=vsb[64:65, :])
                                nc.vector.reciprocal(vsb[64:65, :], vsb[64:65, :])
                                # broadcast reciprocal row to 64 partitions via a
                                # DRAM bounce (SBUF APs need nonzero partition step)
                                rd = dscr.tile([QC], F32, tag="rd",
                                               name=f"rd{i}_{p}_{hh}")
                                nc.sync.dma_start(out=rd.unsqueeze(0), in_=vsb[64:65, :])
                                rbc = work.tile([64, QC], F32, tag="rbc",
                                                name=f"rbc{i}_{p}_{hh}")
                                nc.sync.dma_start(
                                    out=rbc,
                                    in_=bass.AP(tensor=rd.tensor, offset=rd.offset,
                                                ap=[[0, 64]] + list(rd.ap)))
                                # write swizzled: valT_r[hd, j*128 + s'] =
                                # values[hd, s'*16+j]; the later scramble DMA
                                # then reads 256B-contiguous chunks
                                vr = valuesT[h].rearrange("p (j s) -> p j s", j=16)
                                nc.vector.tensor_mul(
                                    vr[:, :, 16 * p:16 * (p + 1)].rearrange("p j s -> p s j"),
                                    vsb[0:64, :].rearrange("p (a b) -> p a b", a=16),
                                    rbc.rearrange("p (a b) -> p a b", a=16))
                        yield final

                    def out_proj_units(h):
                        """out rows = scrVals_h @ W_out via K=128 d'-tiles.
                        vT2[j2*64+hd, t, s'] = values[hd, s'*16 + 2t + j2]
                        gathered from the swizzled valT_r with two DMAs."""
                        vT2 = work.tile([128, NKT, 128], BF16, tag="vT2",
                                        name=f"vT2_{h}")
                        vrr = valuesT[h].rearrange("p (j s) -> p j s", j=16)

                        def scramble():
                            if dbg:
                                nc.gpsimd.dma_start(out=D_VT[h], in_=valuesT[h])
                            for j2 in range(2):
                                nc.sync.dma_start(
                                    out=vT2[j2 * 64:(j2 + 1) * 64, :, :],
                                    in_=vrr[:, j2::2, :])

                        def unit(nh):
                            def emit():
                                if nh == 0:
                                    scramble()
                                # borrow a valT bank (PSUM is fully allocated);
                                # pair h//2 has finished attention by now
                                po = ps_val.tile([128, 512], F32, tag=f"valT{h // 2}",
                                                 name=f"po{h}_{nh}")
                                for t in range(NKT):
                                    nc.tensor.matmul(
                                        po,
                                        vT2[:, t, :],
                                        wo128n[:, t, nh * 512:(nh + 1) * 512],
                                        start=(t == 0), stop=(t == NKT - 1))
                                osb = work.tile([128, 512], F32, tag="osb",
                                                name=f"osb{h}_{nh}")
                                nc.vector.tensor_add(
                                    osb, po, bo_bc[:, nh * 512:(nh + 1) * 512])
                                nc.sync.dma_start(
                                    out=OUT[h * 128:(h + 1) * 128, nh * 512:(nh + 1) * 512],
                                    in_=osb)
                            return emit
                        return [unit(0), unit(1)]

                    # --- fused schedule: pair 0 leads pair 1 by one panel;
                    #     projection / out-projection units fill PE gaps.
                    #     Fillers are tagged with the s-panel they belong to and
                    #     force-flushed before any attention panel that depends
                    #     on them (emission order defines the program order).
                    from collections import deque
                    fill = deque()

                    def pop_fill():
                        if fill:
                            fill.popleft()[1]()

                    def flush_upto(sp):
                        while fill and fill[0][0] <= sp:
                            fill.popleft()[1]()

                    # just enough projection for the first attention panel;
                    # the rest becomes tagged gap-filler
                    proj_qk_unit(0, 0, 0)()
                    proj_qk_unit(0, 0, 1)()
                    fill.extend([(0, proj_qk_unit(0, 1, 0)), (0, proj_qk_unit(0, 1, 1))])
                    fill.extend((0, proj_v_unit(sb)) for sb in range(4))
                    fill.extend((1, u) for u in proj_units(1))
                    emitted_sp = {0, 1}
                    flush_upto(0)  # panel 0 needs vext s-blocks 0..1
                    for st in attn_steps(0, 0):
                        st()
                        pop_fill()
                    for p in range(1, NQP):
                        sp_next = (p + 1) // 2
                        if sp_next <= 3 and sp_next not in emitted_sp:
                            emitted_sp.add(sp_next)
                            fill.extend((sp_next, u) for u in proj_units(sp_next))
                        flush_upto(p // 2)  # kT cols + vext blocks this panel reads
                        g0 = attn_steps(0, p)
                        g1 = attn_steps(1, p - 1)
                        done0 = done1 = False
                        while not (done0 and done1):
                            if not done0:
                                st = next(g0, None)
                                if st is None:
                                    done0 = True
                                else:
                                    st()
                            if not done1:
                                st = next(g1, None)
                                if st is None:
                                    done1 = True
                                else:
                                    st()
                            pop_fill()
                    # drain remaining projection units, if any
                    while fill:
                        fill.popleft()[1]()
                    # pair 1's last panel; pair 0's output projections fill gaps
                    fill.extend((9, u) for u in out_proj_units(0))
                    fill.extend((9, u) for u in out_proj_units(1))
                    for st in attn_steps(1, NQP - 1):
                        st()
                        pop_fill()
                        pop_fill()
                    while fill:
                        fill.popleft()[1]()
                    for u in out_proj_units(2):
                        u()
                    for u in out_proj_units(3):
                        u()

                if dbg:
                    for i in range(2):
                        nc.gpsimd.dma_start(out=D_QT[i], in_=qT[i])
                        nc.gpsimd.dma_start(out=D_KT[i], in_=kT[i])
                    for h in range(HPC):
                        nc.gpsimd.dma_start(out=D_VE[h], in_=vext[h])

    nc.compile()
    return nc


@functools.lru_cache(maxsize=1)
def _get_nc():
    return build_nc()


def kernel(x, W_qkv, b_qkv, W_out, b_out, mask=None, **_unused):
    x = np.asarray(x, dtype=np.float32)
    W_qkv = np.asarray(W_qkv, dtype=np.float32)
    b_qkv = np.asarray(b_qkv, dtype=np.float32)
    W_out = np.asarray(W_out, dtype=np.float32)
    b_out = np.asarray(b_out, dtype=np.float32)

    nc = _get_nc()
    c = np.ascontiguousarray
    # fused QKV layout: head h occupies columns [h*192, (h+1)*192) of W_qkv,
    # as q/k/v sub-blocks of 64 each (reshape(B,S,H,3*HD) then split).
    in_maps = []
    for core in range(NCORES):
        b = core // 4
        hg = core % 4
        heads = [4 * hg + j for j in range(HPC)]
        wq_c = np.concatenate([W_qkv[:, h * 192:h * 192 + 64] for h in heads], axis=1)
        wk_c = np.concatenate([W_qkv[:, h * 192 + 64:h * 192 + 128] for h in heads], axis=1)
        wv_c = np.concatenate([W_qkv[:, h * 192 + 128:h * 192 + 192] for h in heads], axis=1)
        bq_c = np.concatenate([b_qkv[h * 192:h * 192 + 64] for h in heads])
        bk_c = np.concatenate([b_qkv[h * 192 + 64:h * 192 + 128] for h in heads])
        bv_c = np.concatenate([b_qkv[h * 192 + 128:h * 192 + 192] for h in heads])
        in_maps.append({
            "X": c(x[b]),
            "WQ": c(wq_c), "WK": c(wk_c), "WV": c(wv_c),
            "BQ": c(bq_c), "BK": c(bk_c), "BV": c(bv_c),
            "WO": c(W_out),
            "BO": c(b_out),
        })
    global _last_in_maps
    _last_in_maps = in_maps
    res = bass_utils.run_bass_kernel_spmd(nc, in_maps, core_ids=list(range(NCORES)))
    out = np.empty((2, S, D), dtype=np.float32)
    for core in range(NCORES):
        b = core // 4
        hg = core % 4
        out[b, hg * 512:(hg + 1) * 512, :] = res.results[core]["OUT"]
    return out

